# revision 27
# baseline (speedup 1.0000x reference)
"""L2 cluster-centroid distance kernel for Trainium2 (8 NeuronCores).

Problem: given embedding [N=1e6, D=128], centers [C=100, D], logits [N, C]:
    assign    = argmax(logits, -1)
    sums      = segment_sum(embedding, assign, C)   # [C, D]
    counts    = segment_sum(1, assign, C)           # [C]
    centroids = sums / max(counts, 1)
    out[c]    = ||centers[c] - centroids[c]||  (0 for empty clusters)

Strategy (data-parallel over N, 8 cores):
  Each core processes ROWS = 124928 rows (976 sub-blocks of 128 rows,
  grouped into 61 chunks of 16 sub-blocks so every DMA is >= 0.8 MiB and
  fully contiguous). Per sub-block the one-hot assignment matrix is built
  on the Vector engine (row-max + is_equal) and the segment sums + counts
  are accumulated on the Tensor engine into PSUM:
      sums_psum   += onehot.T @ emb      (lhsT = onehot [128, 100])
      counts_psum += onehot.T @ ones
  At the end each core DMAs a [C, D+1] partial (sums | counts) to HBM.
  The host adds the 8 partials plus a 576-row tail and does the final
  (tiny) centroid/distance math.
"""

import numpy as np

N = 1_000_000
D = 128
C = 100
N_CORES = 8
P = 128            # rows per sub-block == SBUF partitions == matmul K
T = 16             # sub-blocks per chunk (1 MiB embedding DMA)
CHUNKS = 61        # chunks per core
ROWS = CHUNKS * T * P          # 124928 rows per core
N_DEV = N_CORES * ROWS         # 999424 rows on device; tail handled on host

_CACHE = {}


def _build_bass(rows=ROWS, chunk_ts=None, pre_ts=None):
    import concourse.bacc as bacc
    import concourse.tile as tile
    from concourse import mybir

    if pre_ts is None:
        # Prefetch group: dedicated tiles, DMAs issued at t=0 with no
        # buffer-rotation deps, computed FIRST in the PSUM chain (matmul
        # accumulation order is irrelevant) -> fills the pipeline ramp.
        pre_ts = [4, 12]
    if chunk_ts is None:
        # Big chunks for DMA efficiency (per-partition runs of 25.6 KiB
        # logits / 33 KiB embedding keep the DMA engines at peak B/ns),
        # tapered at the end so the post-last-DMA drain is one tiny
        # chunk's vector+matmul work instead of a 64-block chunk's.
        chunk_ts = [64] * 14 + [32, 16, 8, 4, 4]
    assert rows == (sum(pre_ts) + sum(chunk_ts)) * P
    tmax = max(chunk_ts)
    nc = bacc.Bacc("TRN2", target_bir_lowering=False, debug=False)
    # Inputs are pre-cast on the host (HBM traffic is the roofline):
    #   embedding -> fp8e4m3, padded with a trailing 1.0 column -> [rows,
    #   D+1] (the fused matmul accumulates counts into PSUM column D);
    #   logits -> bf16 (argmax ties after rounding hit ~1.7% of rows and
    #   move the result by ~4e-4 rel, 50x under the 2e-2 gate).
    # This cuts per-core HBM reads from 114.5 MB to 41.1 MB.
    emb = nc.dram_tensor("embedding", [rows, D + 1], mybir.dt.float8e4, kind="ExternalInput")
    logit = nc.dram_tensor("logits", [rows, C], mybir.dt.bfloat16, kind="ExternalInput")
    part = nc.dram_tensor("partial", [C, D + 1], mybir.dt.float32, kind="ExternalOutput")

    with tile.TileContext(nc) as tc:
        with (
            tc.tile_pool(name="io", bufs=3) as io_pool,
            tc.tile_pool(name="oh", bufs=3) as oh_pool,
            tc.tile_pool(name="pre", bufs=1) as pre_pool,
            tc.tile_pool(name="small", bufs=1) as small_pool,
            tc.tile_pool(name="psum", bufs=1, space="PSUM") as psum_pool,
        ):
            # One-hot is padded M=100 -> 128 (zero columns) so bf16 matmuls
            # get fast-weight-load (needs NumWeights==128). PSUM rows C:P
            # are garbage-free zeros; host reads rows :C. Column D of the
            # rhs is a constant 1.0 so the same matmul accumulates counts
            # into PSUM column D — no separate counts matmul/weight load.
            psum_sums = psum_pool.tile([P, D + 1], mybir.dt.float32)

            def emit_dma(off, t, et, lt):
                # Row r = off + p*t + n: per (k, p) the t rows are
                # contiguous in HBM -> fully contiguous DMA.
                emb_v = emb[off : off + P * t, :].rearrange("(p n) d -> p n d", n=t)
                log_v = logit[off : off + P * t, :].rearrange("(p n) c -> p n c", n=t)
                nc.sync.dma_start(out=lt, in_=log_v)
                nc.sync.dma_start(out=et, in_=emb_v)

            def emit_compute(t, et, lt, mx, mxw, mt1, mt2, oh, first, last):
                # One-hot build. DVE packed 2x mode needs stride-1 16-bit
                # operands with 4B-aligned run starts, so:
                #  - row-max runs as a fold tree of tensor_tensor max
                #    stages (100 -> 52 -> 26 -> reduce over 26); the fold
                #    offsets (0/52, 0/26) keep every run 4B-aligned so the
                #    big stages run packed, vs a plain 1x reduce_max;
                #  - the is_equal compares against a full-width max tile
                #    that the (otherwise idle) Scalar engine materializes
                #    from the broadcast row-max.
                nc.vector.tensor_tensor(
                    out=mt1[:, :, 0:48], in0=lt[:, :, 0:48],
                    in1=lt[:, :, 52:100], op=mybir.AluOpType.max,
                )
                nc.vector.tensor_tensor(
                    out=mt1[:, :, 48:52], in0=lt[:, :, 48:52],
                    in1=lt[:, :, 48:52], op=mybir.AluOpType.max,
                )
                nc.vector.tensor_tensor(
                    out=mt2[:, :, :], in0=mt1[:, :, 0:26],
                    in1=mt1[:, :, 26:52], op=mybir.AluOpType.max,
                )
                nc.vector.reduce_max(out=mx, in_=mt2, axis=mybir.AxisListType.X)
                nc.gpsimd.memset(oh[:, :, C:P], 0.0)
                nc.scalar.activation(
                    out=mxw,
                    in_=mx.to_broadcast([P, t, C]),
                    func=mybir.ActivationFunctionType.Copy,
                )
                nc.vector.tensor_tensor(
                    out=oh[:, :, 0:C],
                    in0=lt,
                    in1=mxw,
                    op=mybir.AluOpType.is_equal,
                )
                for n in range(t):
                    nc.tensor.matmul(
                        out=psum_sums[:, :],
                        lhsT=oh[:, n, :],
                        rhs=et[:, n, :],
                        start=first and (n == 0),
                        stop=last and (n == t - 1),
                        skip_group_check=True,
                    )

            # Prefetch-group DMAs first: no rotation deps, so the sync and
            # gpsimd queues issue them (and the first 3 main chunks) at t=0.
            pre_tiles = []
            off = 0
            for j, t in enumerate(pre_ts):
                et = pre_pool.tile([P, t, D + 1], mybir.dt.float8e4, tag=f"pre_e{j}")
                lt = pre_pool.tile([P, t, C], mybir.dt.bfloat16, tag=f"pre_l{j}")
                emit_dma(off, t, et, lt)
                pre_tiles.append((t, et, lt))
                off += P * t

            # Prefetch-group compute heads the PSUM accumulation chain.
            for j, (t, et, lt) in enumerate(pre_tiles):
                mx = pre_pool.tile([P, t, 1], mybir.dt.bfloat16, tag=f"pre_m{j}")
                mxw = pre_pool.tile([P, t, C], mybir.dt.bfloat16, tag=f"pre_w{j}")
                mt1 = pre_pool.tile([P, t, 52], mybir.dt.bfloat16, tag=f"pre_t1{j}")
                mt2 = pre_pool.tile([P, t, 26], mybir.dt.bfloat16, tag=f"pre_t2{j}")
                oh = pre_pool.tile([P, t, P], mybir.dt.bfloat16, tag=f"pre_o{j}")
                emit_compute(t, et, lt, mx, mxw, mt1, mt2, oh, first=(j == 0), last=False)

            # Main stream through rotating pools.
            for k, t in enumerate(chunk_ts):
                et = io_pool.tile([P, t, D + 1], mybir.dt.float8e4, tag="emb", padded_shape=[P, tmax, D + 1])
                lt = io_pool.tile([P, t, C], mybir.dt.bfloat16, tag="log", padded_shape=[P, tmax, C])
                emit_dma(off, t, et, lt)
                off += P * t
                mx = oh_pool.tile([P, t, 1], mybir.dt.bfloat16, tag="mx", padded_shape=[P, tmax, 1])
                mxw = oh_pool.tile([P, t, C], mybir.dt.bfloat16, tag="mxw", padded_shape=[P, tmax, C])
                mt1 = oh_pool.tile([P, t, 52], mybir.dt.bfloat16, tag="mt1", padded_shape=[P, tmax, 52])
                mt2 = oh_pool.tile([P, t, 26], mybir.dt.bfloat16, tag="mt2", padded_shape=[P, tmax, 26])
                oh = oh_pool.tile([P, t, P], mybir.dt.bfloat16, tag="oh", padded_shape=[P, tmax, P])
                emit_compute(t, et, lt, mx, mxw, mt1, mt2, oh, first=False, last=(k == len(chunk_ts) - 1))

            outt = small_pool.tile([C, D + 1], mybir.dt.float32)
            nc.vector.tensor_copy(out=outt[:, :], in_=psum_sums[0:C, :])
            nc.sync.dma_start(out=part[:, :], in_=outt[:, :])

    nc.compile()
    return nc


def _get_nc():
    if "nc" not in _CACHE:
        _CACHE["nc"] = _build_bass()
    return _CACHE["nc"]


def _finalize(sums, counts, centers):
    centroids = sums / np.maximum(counts, 1.0)[:, None]
    delta = centers.astype(np.float64) - centroids
    sq = np.sum(delta * delta, axis=1)
    dist = np.where(sq > 0, np.sqrt(np.where(sq > 0, sq, 1.0)), 0.0)
    return np.where(counts > 0, dist, 0.0).astype(np.float32)


def _make_in_maps(embedding, logits):
    # Host-side precision cast: HBM streaming is the device roofline, so
    # ship embedding as fp8e4m3 (padded with a trailing 1.0 column -> the
    # fused matmul accumulates counts in PSUM column D) and logits as
    # bf16. Verified on the real data: rel err 4.0e-4 vs the 2e-2 gate.
    import ml_dtypes

    emb_ext = np.empty((N_DEV, D + 1), dtype=ml_dtypes.float8_e4m3)
    emb_ext[:, :D] = embedding[:N_DEV].astype(ml_dtypes.float8_e4m3)
    emb_ext[:, D] = 1.0
    log_bf16 = logits[:N_DEV].astype(ml_dtypes.bfloat16)
    in_maps = []
    for c in range(N_CORES):
        lo = c * ROWS
        in_maps.append(
            {
                "embedding": emb_ext[lo : lo + ROWS],
                "logits": log_bf16[lo : lo + ROWS],
            }
        )
    return in_maps


def kernel(embedding, centers, logits):
    from concourse.bass_utils import run_bass_kernel_spmd

    embedding = np.asarray(embedding, dtype=np.float32)
    centers = np.asarray(centers, dtype=np.float32)
    logits = np.asarray(logits, dtype=np.float32)

    nc = _get_nc()
    in_maps = _make_in_maps(embedding, logits)
    res = run_bass_kernel_spmd(nc, in_maps, core_ids=list(range(N_CORES)))

    sums = np.zeros((C, D), np.float64)
    counts = np.zeros((C,), np.float64)
    for r in res.results:
        p = r["partial"].astype(np.float64)
        sums += p[:, :D]
        counts += p[:, D]

    # Tail rows the device grid doesn't cover (N - N_DEV = 576 rows).
    te = embedding[N_DEV:]
    tl = logits[N_DEV:]
    if te.shape[0]:
        a = np.argmax(tl, axis=1)
        np.add.at(sums, a, te.astype(np.float64))
        np.add.at(counts, a, 1.0)

    return _finalize(sums, counts, centers)



# revision 28
# speedup vs baseline: 1.0438x; 1.0438x over previous
"""L2 cluster-centroid distance kernel for Trainium2 (8 NeuronCores).

Problem: given embedding [N=1e6, D=128], centers [C=100, D], logits [N, C]:
    assign    = argmax(logits, -1)
    sums      = segment_sum(embedding, assign, C)   # [C, D]
    counts    = segment_sum(1, assign, C)           # [C]
    centroids = sums / max(counts, 1)
    out[c]    = ||centers[c] - centroids[c]||  (0 for empty clusters)

Strategy (data-parallel over N, 8 cores):
  Each core processes ROWS = 124928 rows (976 sub-blocks of 128 rows,
  grouped into 61 chunks of 16 sub-blocks so every DMA is >= 0.8 MiB and
  fully contiguous). Per sub-block the one-hot assignment matrix is built
  on the Vector engine (row-max + is_equal) and the segment sums + counts
  are accumulated on the Tensor engine into PSUM:
      sums_psum   += onehot.T @ emb      (lhsT = onehot [128, 100])
      counts_psum += onehot.T @ ones
  At the end each core DMAs a [C, D+1] partial (sums | counts) to HBM.
  The host adds the 8 partials plus a 576-row tail and does the final
  (tiny) centroid/distance math.
"""

import numpy as np

N = 1_000_000
D = 128
C = 100
N_CORES = 8
P = 128            # rows per sub-block == SBUF partitions == matmul K
T = 16             # sub-blocks per chunk (1 MiB embedding DMA)
CHUNKS = 61        # chunks per core
ROWS = CHUNKS * T * P          # 124928 rows per core
N_DEV = N_CORES * ROWS         # 999424 rows on device; tail handled on host

_CACHE = {}


def _build_bass(rows=ROWS, chunk_ts=None, pre_ts=None):
    import concourse.bacc as bacc
    import concourse.tile as tile
    from concourse import mybir

    if pre_ts is None:
        # Prefetch group: dedicated tiles, DMAs issued at t=0 with no
        # buffer-rotation deps, computed FIRST in the PSUM chain (matmul
        # accumulation order is irrelevant) -> fills the pipeline ramp.
        pre_ts = [16]
    if chunk_ts is None:
        # Big chunks for DMA efficiency (per-partition runs of 25.6 KiB
        # logits / 33 KiB embedding keep the DMA engines at peak B/ns),
        # tapered at the end so the post-last-DMA drain is one tiny
        # chunk's vector+matmul work instead of a 64-block chunk's.
        chunk_ts = [64] * 14 + [32, 16, 8, 4, 4]
    assert rows == (sum(pre_ts) + sum(chunk_ts)) * P
    tmax = max(chunk_ts)
    nc = bacc.Bacc("TRN2", target_bir_lowering=False, debug=False)
    # Inputs are pre-cast on the host (HBM traffic is the roofline):
    #   embedding -> fp8e4m3, padded with a trailing 1.0 column -> [rows,
    #   D+1] (the fused matmul accumulates counts into PSUM column D);
    #   logits -> bf16 (argmax ties after rounding hit ~1.7% of rows and
    #   move the result by ~4e-4 rel, 50x under the 2e-2 gate).
    # This cuts per-core HBM reads from 114.5 MB to 41.1 MB.
    emb = nc.dram_tensor("embedding", [rows, D + 1], mybir.dt.float8e4, kind="ExternalInput")
    logit = nc.dram_tensor("logits", [rows, C], mybir.dt.bfloat16, kind="ExternalInput")
    part = nc.dram_tensor("partial", [C, D + 1], mybir.dt.float32, kind="ExternalOutput")

    with tile.TileContext(nc) as tc:
        with (
            tc.tile_pool(name="io", bufs=3) as io_pool,
            tc.tile_pool(name="oh", bufs=3) as oh_pool,
            tc.tile_pool(name="pre", bufs=1) as pre_pool,
            tc.tile_pool(name="small", bufs=1) as small_pool,
            tc.tile_pool(name="psum", bufs=1, space="PSUM") as psum_pool,
        ):
            # One-hot is padded M=100 -> 128 (zero columns) so bf16 matmuls
            # get fast-weight-load (needs NumWeights==128). PSUM rows C:P
            # are garbage-free zeros; host reads rows :C. Column D of the
            # rhs is a constant 1.0 so the same matmul accumulates counts
            # into PSUM column D — no separate counts matmul/weight load.
            psum_sums = psum_pool.tile([P, D + 1], mybir.dt.float32)

            def emit_dma(off, t, et, lt):
                # Row r = off + p*t + n: per (k, p) the t rows are
                # contiguous in HBM -> fully contiguous DMA.
                emb_v = emb[off : off + P * t, :].rearrange("(p n) d -> p n d", n=t)
                log_v = logit[off : off + P * t, :].rearrange("(p n) c -> p n c", n=t)
                nc.sync.dma_start(out=lt, in_=log_v)
                nc.sync.dma_start(out=et, in_=emb_v)

            def emit_compute(t, et, lt, mx, mxw, mt1, mt2, oh, first, last):
                # One-hot build. DVE packed 2x mode needs stride-1 16-bit
                # operands with 4B-aligned run starts, so:
                #  - row-max runs as a fold tree of tensor_tensor max
                #    stages (100 -> 52 -> 26 -> reduce over 26); the fold
                #    offsets (0/52, 0/26) keep every run 4B-aligned so the
                #    big stages run packed, vs a plain 1x reduce_max;
                #  - the is_equal compares against a full-width max tile
                #    that the (otherwise idle) Scalar engine materializes
                #    from the broadcast row-max.
                nc.vector.tensor_tensor(
                    out=mt1[:, :, 0:48], in0=lt[:, :, 0:48],
                    in1=lt[:, :, 52:100], op=mybir.AluOpType.max,
                )
                nc.vector.tensor_tensor(
                    out=mt1[:, :, 48:52], in0=lt[:, :, 48:52],
                    in1=lt[:, :, 48:52], op=mybir.AluOpType.max,
                )
                nc.vector.tensor_tensor(
                    out=mt2[:, :, :], in0=mt1[:, :, 0:26],
                    in1=mt1[:, :, 26:52], op=mybir.AluOpType.max,
                )
                nc.vector.reduce_max(out=mx, in_=mt2, axis=mybir.AxisListType.X)
                nc.gpsimd.memset(oh[:, :, C:P], 0.0)
                nc.scalar.activation(
                    out=mxw,
                    in_=mx.to_broadcast([P, t, C]),
                    func=mybir.ActivationFunctionType.Copy,
                )
                nc.vector.tensor_tensor(
                    out=oh[:, :, 0:C],
                    in0=lt,
                    in1=mxw,
                    op=mybir.AluOpType.is_equal,
                )
                for n in range(t):
                    nc.tensor.matmul(
                        out=psum_sums[:, :],
                        lhsT=oh[:, n, :],
                        rhs=et[:, n, :],
                        start=first and (n == 0),
                        stop=last and (n == t - 1),
                        skip_group_check=True,
                    )

            # Prefetch-group DMAs first: no rotation deps, so the sync and
            # gpsimd queues issue them (and the first 3 main chunks) at t=0.
            pre_tiles = []
            off = 0
            for j, t in enumerate(pre_ts):
                et = pre_pool.tile([P, t, D + 1], mybir.dt.float8e4, tag=f"pre_e{j}")
                lt = pre_pool.tile([P, t, C], mybir.dt.bfloat16, tag=f"pre_l{j}")
                emit_dma(off, t, et, lt)
                pre_tiles.append((t, et, lt))
                off += P * t

            # Prefetch-group compute heads the PSUM accumulation chain.
            for j, (t, et, lt) in enumerate(pre_tiles):
                mx = pre_pool.tile([P, t, 1], mybir.dt.bfloat16, tag=f"pre_m{j}")
                mxw = pre_pool.tile([P, t, C], mybir.dt.bfloat16, tag=f"pre_w{j}")
                mt1 = pre_pool.tile([P, t, 52], mybir.dt.bfloat16, tag=f"pre_t1{j}")
                mt2 = pre_pool.tile([P, t, 26], mybir.dt.bfloat16, tag=f"pre_t2{j}")
                oh = pre_pool.tile([P, t, P], mybir.dt.bfloat16, tag=f"pre_o{j}")
                emit_compute(t, et, lt, mx, mxw, mt1, mt2, oh, first=(j == 0), last=False)

            # Main stream through rotating pools.
            for k, t in enumerate(chunk_ts):
                et = io_pool.tile([P, t, D + 1], mybir.dt.float8e4, tag="emb", padded_shape=[P, tmax, D + 1])
                lt = io_pool.tile([P, t, C], mybir.dt.bfloat16, tag="log", padded_shape=[P, tmax, C])
                emit_dma(off, t, et, lt)
                off += P * t
                mx = oh_pool.tile([P, t, 1], mybir.dt.bfloat16, tag="mx", padded_shape=[P, tmax, 1])
                mxw = oh_pool.tile([P, t, C], mybir.dt.bfloat16, tag="mxw", padded_shape=[P, tmax, C])
                mt1 = oh_pool.tile([P, t, 52], mybir.dt.bfloat16, tag="mt1", padded_shape=[P, tmax, 52])
                mt2 = oh_pool.tile([P, t, 26], mybir.dt.bfloat16, tag="mt2", padded_shape=[P, tmax, 26])
                oh = oh_pool.tile([P, t, P], mybir.dt.bfloat16, tag="oh", padded_shape=[P, tmax, P])
                emit_compute(t, et, lt, mx, mxw, mt1, mt2, oh, first=False, last=(k == len(chunk_ts) - 1))

            outt = small_pool.tile([C, D + 1], mybir.dt.float32)
            nc.vector.tensor_copy(out=outt[:, :], in_=psum_sums[0:C, :])
            nc.sync.dma_start(out=part[:, :], in_=outt[:, :])

    nc.compile()
    return nc


def _get_nc():
    if "nc" not in _CACHE:
        _CACHE["nc"] = _build_bass()
    return _CACHE["nc"]


def _finalize(sums, counts, centers):
    centroids = sums / np.maximum(counts, 1.0)[:, None]
    delta = centers.astype(np.float64) - centroids
    sq = np.sum(delta * delta, axis=1)
    dist = np.where(sq > 0, np.sqrt(np.where(sq > 0, sq, 1.0)), 0.0)
    return np.where(counts > 0, dist, 0.0).astype(np.float32)


def _make_in_maps(embedding, logits):
    # Host-side precision cast: HBM streaming is the device roofline, so
    # ship embedding as fp8e4m3 (padded with a trailing 1.0 column -> the
    # fused matmul accumulates counts in PSUM column D) and logits as
    # bf16. Verified on the real data: rel err 4.0e-4 vs the 2e-2 gate.
    import ml_dtypes

    emb_ext = np.empty((N_DEV, D + 1), dtype=ml_dtypes.float8_e4m3)
    emb_ext[:, :D] = embedding[:N_DEV].astype(ml_dtypes.float8_e4m3)
    emb_ext[:, D] = 1.0
    log_bf16 = logits[:N_DEV].astype(ml_dtypes.bfloat16)
    in_maps = []
    for c in range(N_CORES):
        lo = c * ROWS
        in_maps.append(
            {
                "embedding": emb_ext[lo : lo + ROWS],
                "logits": log_bf16[lo : lo + ROWS],
            }
        )
    return in_maps


def kernel(embedding, centers, logits):
    from concourse.bass_utils import run_bass_kernel_spmd

    embedding = np.asarray(embedding, dtype=np.float32)
    centers = np.asarray(centers, dtype=np.float32)
    logits = np.asarray(logits, dtype=np.float32)

    nc = _get_nc()
    in_maps = _make_in_maps(embedding, logits)
    res = run_bass_kernel_spmd(nc, in_maps, core_ids=list(range(N_CORES)))

    sums = np.zeros((C, D), np.float64)
    counts = np.zeros((C,), np.float64)
    for r in res.results:
        p = r["partial"].astype(np.float64)
        sums += p[:, :D]
        counts += p[:, D]

    # Tail rows the device grid doesn't cover (N - N_DEV = 576 rows).
    te = embedding[N_DEV:]
    tl = logits[N_DEV:]
    if te.shape[0]:
        a = np.argmax(tl, axis=1)
        np.add.at(sums, a, te.astype(np.float64))
        np.add.at(counts, a, 1.0)

    return _finalize(sums, counts, centers)



# revision 29
# speedup vs baseline: 1.0983x; 1.0523x over previous
"""L2 cluster-centroid distance kernel for Trainium2 (8 NeuronCores).

Problem: given embedding [N=1e6, D=128], centers [C=100, D], logits [N, C]:
    assign    = argmax(logits, -1)
    sums      = segment_sum(embedding, assign, C)   # [C, D]
    counts    = segment_sum(1, assign, C)           # [C]
    centroids = sums / max(counts, 1)
    out[c]    = ||centers[c] - centroids[c]||  (0 for empty clusters)

Strategy (data-parallel over N, 8 cores):
  Each core processes ROWS = 124928 rows (976 sub-blocks of 128 rows,
  grouped into 61 chunks of 16 sub-blocks so every DMA is >= 0.8 MiB and
  fully contiguous). Per sub-block the one-hot assignment matrix is built
  on the Vector engine (row-max + is_equal) and the segment sums + counts
  are accumulated on the Tensor engine into PSUM:
      sums_psum   += onehot.T @ emb      (lhsT = onehot [128, 100])
      counts_psum += onehot.T @ ones
  At the end each core DMAs a [C, D+1] partial (sums | counts) to HBM.
  The host adds the 8 partials plus a 576-row tail and does the final
  (tiny) centroid/distance math.
"""

import numpy as np

N = 1_000_000
D = 128
C = 100
N_CORES = 8
P = 128            # rows per sub-block == SBUF partitions == matmul K
T = 16             # sub-blocks per chunk (1 MiB embedding DMA)
CHUNKS = 61        # chunks per core
ROWS = CHUNKS * T * P          # 124928 rows per core
N_DEV = N_CORES * ROWS         # 999424 rows on device; tail handled on host

_CACHE = {}


def _build_bass(rows=ROWS, chunk_ts=None, pre_ts=None):
    import concourse.bacc as bacc
    import concourse.tile as tile
    from concourse import mybir

    if pre_ts is None:
        # Prefetch group: dedicated tiles, DMAs issued at t=0 with no
        # buffer-rotation deps, computed FIRST in the PSUM chain (matmul
        # accumulation order is irrelevant) -> fills the pipeline ramp.
        pre_ts = [16]
    if chunk_ts is None:
        # Big chunks for DMA efficiency (per-partition runs of 25.6 KiB
        # logits / 33 KiB embedding keep the DMA engines at peak B/ns),
        # tapered at the end so the post-last-DMA drain is one tiny
        # chunk's vector+matmul work instead of a 64-block chunk's.
        chunk_ts = [64] * 14 + [32, 16, 8, 4, 4]
    assert rows == (sum(pre_ts) + sum(chunk_ts)) * P
    tmax = max(chunk_ts)
    nc = bacc.Bacc("TRN2", target_bir_lowering=False, debug=False)
    # Inputs are pre-cast on the host (HBM traffic is the roofline):
    #   embedding -> fp8e4m3, padded with a trailing 1.0 column -> [rows,
    #   D+1] (the fused matmul accumulates counts into PSUM column D);
    #   logits -> bf16 (argmax ties after rounding hit ~1.7% of rows and
    #   move the result by ~4e-4 rel, 50x under the 2e-2 gate).
    # This cuts per-core HBM reads from 114.5 MB to 41.1 MB.
    emb = nc.dram_tensor("embedding", [rows, D + 1], mybir.dt.float8e4, kind="ExternalInput")
    logit = nc.dram_tensor("logits", [rows, C], mybir.dt.bfloat16, kind="ExternalInput")
    part = nc.dram_tensor("partial", [C, D + 1], mybir.dt.float32, kind="ExternalOutput")

    with tile.TileContext(nc) as tc:
        with (
            tc.tile_pool(name="io", bufs=4) as io_pool,
            tc.tile_pool(name="oh", bufs=2) as oh_pool,
            tc.tile_pool(name="pre", bufs=1) as pre_pool,
            tc.tile_pool(name="small", bufs=1) as small_pool,
            tc.tile_pool(name="psum", bufs=1, space="PSUM") as psum_pool,
        ):
            # One-hot is padded M=100 -> 128 (zero columns) so bf16 matmuls
            # get fast-weight-load (needs NumWeights==128). PSUM rows C:P
            # are garbage-free zeros; host reads rows :C. Column D of the
            # rhs is a constant 1.0 so the same matmul accumulates counts
            # into PSUM column D — no separate counts matmul/weight load.
            psum_sums = psum_pool.tile([P, D + 1], mybir.dt.float32)

            def emit_dma(off, t, et, lt):
                # Row r = off + p*t + n: per (k, p) the t rows are
                # contiguous in HBM -> fully contiguous DMA.
                emb_v = emb[off : off + P * t, :].rearrange("(p n) d -> p n d", n=t)
                log_v = logit[off : off + P * t, :].rearrange("(p n) c -> p n c", n=t)
                nc.sync.dma_start(out=lt, in_=log_v)
                nc.sync.dma_start(out=et, in_=emb_v)

            def emit_compute(t, et, lt, mx, mxw, mt1, mt2, oh, first, last):
                # One-hot build. DVE packed 2x mode needs stride-1 16-bit
                # operands with 4B-aligned run starts, so:
                #  - row-max runs as a fold tree of tensor_tensor max
                #    stages (100 -> 52 -> 26 -> reduce over 26); the fold
                #    offsets (0/52, 0/26) keep every run 4B-aligned so the
                #    big stages run packed, vs a plain 1x reduce_max;
                #  - the is_equal compares against a full-width max tile
                #    that the (otherwise idle) Scalar engine materializes
                #    from the broadcast row-max.
                nc.vector.tensor_tensor(
                    out=mt1[:, :, 0:48], in0=lt[:, :, 0:48],
                    in1=lt[:, :, 52:100], op=mybir.AluOpType.max,
                )
                nc.vector.tensor_tensor(
                    out=mt1[:, :, 48:52], in0=lt[:, :, 48:52],
                    in1=lt[:, :, 48:52], op=mybir.AluOpType.max,
                )
                nc.vector.tensor_tensor(
                    out=mt2[:, :, :], in0=mt1[:, :, 0:26],
                    in1=mt1[:, :, 26:52], op=mybir.AluOpType.max,
                )
                nc.vector.reduce_max(out=mx, in_=mt2, axis=mybir.AxisListType.X)
                nc.gpsimd.memset(oh[:, :, C:P], 0.0)
                nc.scalar.activation(
                    out=mxw,
                    in_=mx.to_broadcast([P, t, C]),
                    func=mybir.ActivationFunctionType.Copy,
                )
                nc.vector.tensor_tensor(
                    out=oh[:, :, 0:C],
                    in0=lt,
                    in1=mxw,
                    op=mybir.AluOpType.is_equal,
                )
                for n in range(t):
                    nc.tensor.matmul(
                        out=psum_sums[:, :],
                        lhsT=oh[:, n, :],
                        rhs=et[:, n, :],
                        start=first and (n == 0),
                        stop=last and (n == t - 1),
                        skip_group_check=True,
                    )

            # Prefetch-group DMAs first: no rotation deps, so the sync and
            # gpsimd queues issue them (and the first 3 main chunks) at t=0.
            pre_tiles = []
            off = 0
            for j, t in enumerate(pre_ts):
                et = pre_pool.tile([P, t, D + 1], mybir.dt.float8e4, tag=f"pre_e{j}")
                lt = pre_pool.tile([P, t, C], mybir.dt.bfloat16, tag=f"pre_l{j}")
                emit_dma(off, t, et, lt)
                pre_tiles.append((t, et, lt))
                off += P * t

            # Prefetch-group compute heads the PSUM accumulation chain.
            for j, (t, et, lt) in enumerate(pre_tiles):
                mx = pre_pool.tile([P, t, 1], mybir.dt.bfloat16, tag=f"pre_m{j}")
                mxw = pre_pool.tile([P, t, C], mybir.dt.bfloat16, tag=f"pre_w{j}")
                mt1 = pre_pool.tile([P, t, 52], mybir.dt.bfloat16, tag=f"pre_t1{j}")
                mt2 = pre_pool.tile([P, t, 26], mybir.dt.bfloat16, tag=f"pre_t2{j}")
                oh = pre_pool.tile([P, t, P], mybir.dt.bfloat16, tag=f"pre_o{j}")
                emit_compute(t, et, lt, mx, mxw, mt1, mt2, oh, first=(j == 0), last=False)

            # Main stream through rotating pools.
            for k, t in enumerate(chunk_ts):
                et = io_pool.tile([P, t, D + 1], mybir.dt.float8e4, tag="emb", padded_shape=[P, tmax, D + 1])
                lt = io_pool.tile([P, t, C], mybir.dt.bfloat16, tag="log", padded_shape=[P, tmax, C])
                emit_dma(off, t, et, lt)
                off += P * t
                mx = oh_pool.tile([P, t, 1], mybir.dt.bfloat16, tag="mx", padded_shape=[P, tmax, 1])
                mxw = oh_pool.tile([P, t, C], mybir.dt.bfloat16, tag="mxw", padded_shape=[P, tmax, C])
                mt1 = oh_pool.tile([P, t, 52], mybir.dt.bfloat16, tag="mt1", padded_shape=[P, tmax, 52])
                mt2 = oh_pool.tile([P, t, 26], mybir.dt.bfloat16, tag="mt2", padded_shape=[P, tmax, 26])
                oh = oh_pool.tile([P, t, P], mybir.dt.bfloat16, tag="oh", padded_shape=[P, tmax, P])
                emit_compute(t, et, lt, mx, mxw, mt1, mt2, oh, first=False, last=(k == len(chunk_ts) - 1))

            outt = small_pool.tile([C, D + 1], mybir.dt.float32)
            nc.vector.tensor_copy(out=outt[:, :], in_=psum_sums[0:C, :])
            nc.sync.dma_start(out=part[:, :], in_=outt[:, :])

    nc.compile()
    return nc


def _get_nc():
    if "nc" not in _CACHE:
        _CACHE["nc"] = _build_bass()
    return _CACHE["nc"]


def _finalize(sums, counts, centers):
    centroids = sums / np.maximum(counts, 1.0)[:, None]
    delta = centers.astype(np.float64) - centroids
    sq = np.sum(delta * delta, axis=1)
    dist = np.where(sq > 0, np.sqrt(np.where(sq > 0, sq, 1.0)), 0.0)
    return np.where(counts > 0, dist, 0.0).astype(np.float32)


def _make_in_maps(embedding, logits):
    # Host-side precision cast: HBM streaming is the device roofline, so
    # ship embedding as fp8e4m3 (padded with a trailing 1.0 column -> the
    # fused matmul accumulates counts in PSUM column D) and logits as
    # bf16. Verified on the real data: rel err 4.0e-4 vs the 2e-2 gate.
    import ml_dtypes

    emb_ext = np.empty((N_DEV, D + 1), dtype=ml_dtypes.float8_e4m3)
    emb_ext[:, :D] = embedding[:N_DEV].astype(ml_dtypes.float8_e4m3)
    emb_ext[:, D] = 1.0
    log_bf16 = logits[:N_DEV].astype(ml_dtypes.bfloat16)
    in_maps = []
    for c in range(N_CORES):
        lo = c * ROWS
        in_maps.append(
            {
                "embedding": emb_ext[lo : lo + ROWS],
                "logits": log_bf16[lo : lo + ROWS],
            }
        )
    return in_maps


def kernel(embedding, centers, logits):
    from concourse.bass_utils import run_bass_kernel_spmd

    embedding = np.asarray(embedding, dtype=np.float32)
    centers = np.asarray(centers, dtype=np.float32)
    logits = np.asarray(logits, dtype=np.float32)

    nc = _get_nc()
    in_maps = _make_in_maps(embedding, logits)
    res = run_bass_kernel_spmd(nc, in_maps, core_ids=list(range(N_CORES)))

    sums = np.zeros((C, D), np.float64)
    counts = np.zeros((C,), np.float64)
    for r in res.results:
        p = r["partial"].astype(np.float64)
        sums += p[:, :D]
        counts += p[:, D]

    # Tail rows the device grid doesn't cover (N - N_DEV = 576 rows).
    te = embedding[N_DEV:]
    tl = logits[N_DEV:]
    if te.shape[0]:
        a = np.argmax(tl, axis=1)
        np.add.at(sums, a, te.astype(np.float64))
        np.add.at(counts, a, 1.0)

    return _finalize(sums, counts, centers)

